# revision 7
# baseline (speedup 1.0000x reference)
"""GCN VGAE encoder (two GCNConv layers -> (mu, logstd)) on 8 Trainium2
NeuronCores via Bass/Tile.

Math: with deg = 1 + in_degree, dinv = deg^-1/2, and segment aggregation
S(u)[i] = sum_{e: dst e = i} u[src e]:
    u1 = (x @ W1) * dinv           h  = relu(dinv * (S(u1) + u1) + b1)
    u2 = h * dinv                  t  = dinv * (S(u2) + u2)
    mu = t @ W_mu + b_mu           ls = t @ W_ls + b_ls
(the linear transform commutes with segment_sum, so mu/ls share one S pass).

Distribution: nodes (and their incoming edges) sharded over 8 cores by dst
range; u1/u2 AllGathered so every core can gather arbitrary src rows.
Aggregation: edges bucketed by 128 dst rows; per 128-edge chunk a one-hot
selection matrix M[e,d] = (dstrel e == d) is built on DVE and the bucket's
PSUM accumulates M.T @ gathered_rows on the PE.
"""
import numpy as np

import concourse.bass as bass
import concourse.tile as tile
from concourse import mybir
from bass_rust import ScopedClock, SyncInfo

N_NODES = 100000
N_EDGES = 1600000
IN_CH, HID_CH, OUT_CH = 256, 64, 32
N_CORES = 8
NL = N_NODES // N_CORES          # 12500 nodes per core
NB = (NL + 127) // 128           # 98 dst buckets per core
LAST_ROWS = NL - (NB - 1) * 128  # rows in last bucket (84)
XT_COLS = NB * 128               # x^T padded to full tiles (12544)

# ---------------------------------------------------------------------------
# Workarounds for the walrus build in this container: it encodes at most ONE
# semaphore wait per instruction and rejects InstIncSwdgeSem.
# ---------------------------------------------------------------------------
_counter = [0]


def _patched_drain_and_barrier(self, tick_clock, wait_clock):
    drain_inst = self.nc.vector.drain()
    wait_clock.add_sem_waits(
        drain_inst.ins, ScopedClock({None: tick_clock.global_clock})
    )
    waits = list(drain_inst.ins.sync_info.on_wait)
    if len(waits) > 1:
        drain_inst.ins.sync_info = SyncInfo(on_wait=[waits[0]], on_update=[])
        for w in waits[1:]:
            extra = self.nc.vector.drain()
            extra.ins.sync_info = SyncInfo(on_wait=[w], on_update=[])
    self.nc.all_engine_barrier()
    assert self.sems is not None
    popped = self.nc._tile_sem_poison_stack.pop()
    assert popped is self._sem_poison
    self.nc.clear_and_free_semaphores(list(self.sems.allocated().values()))
    self.nc.all_engine_barrier()


tile.TileContext._drain_and_barrier = _patched_drain_and_barrier


def _steal_sem_clear(nc, first, last):
    cur = nc.cur_bb.bb
    inst = nc.gpsimd.sem_clear(range(first, last + 1))
    il = cur.instructions
    assert il and il[-1] is inst.ins
    cur.instructions = il[:-1]
    return inst.ins


def _fix_incswdge(nc):
    for bb in nc.main_func.blocks:
        il = bb.instructions
        if not any(type(i).__name__ == "InstIncSwdgeSem" for i in il):
            continue
        new_list = []
        for ins in il:
            if type(ins).__name__ != "InstIncSwdgeSem":
                new_list.append(ins)
                continue
            base = ins._sem_id_base
            values = list(ins._sem_values)
            names = list(ins._sem_names)
            si = ins.sync_info
            waits = list(si.on_wait) if si is not None else []
            for w in waits:
                _counter[0] += 1
                nop = mybir.InstNoOp(name=f"SWF-{_counter[0]}", ins=[], outs=[])
                nop.engine = ins.engine
                nop.sync_info = SyncInfo(on_wait=[w], on_update=[])
                new_list.append(nop)
            if ins._mode == "sub":
                nz = [k for k, v in enumerate(values) if v]
                if nz:
                    new_list.append(_steal_sem_clear(nc, base + min(nz), base + max(nz)))
            else:
                for k, v in enumerate(values):
                    for _ in range(v):
                        _counter[0] += 1
                        nop = mybir.InstNoOp(name=f"SWF-{_counter[0]}", ins=[], outs=[])
                        nop.engine = ins.engine
                        nop.sync_info = SyncInfo(
                            on_wait=[],
                            on_update=[
                                mybir.SyncUpdate(
                                    sync_type="semaphore", id=base + k,
                                    ant_name=names[k], update_mode="sem-inc",
                                    update_value=v * 0 + 1,
                                )
                            ],
                        )
                        new_list.append(nop)
        bb.instructions = new_list


def _split_multiwaits(nc):
    for bb in nc.main_func.blocks:
        il = bb.instructions
        if not any(
            i.sync_info is not None and len(i.sync_info.on_wait) > 1 for i in il
        ):
            continue
        new_list = []
        for ins in il:
            si = ins.sync_info
            waits = list(si.on_wait) if si is not None else []
            if len(waits) > 1:
                ups = list(si.on_update)
                for w in waits[:-1]:
                    _counter[0] += 1
                    nop = mybir.InstNoOp(name=f"WSP-{_counter[0]}", ins=[], outs=[])
                    nop.engine = ins.engine
                    nop.sync_info = SyncInfo(on_wait=[w], on_update=[])
                    new_list.append(nop)
                ins.sync_info = SyncInfo(on_wait=[waits[-1]], on_update=ups)
            new_list.append(ins)
        bb.instructions = new_list


# ---------------------------------------------------------------------------
# Device program
# ---------------------------------------------------------------------------
def _build_program(cpb):
    f32, i32 = mybir.dt.float32, mybir.dt.int32
    nc = bass.Bass("TRN2", target_bir_lowering=False, debug=False,
                   num_devices=N_CORES)
    C = NB * cpb  # chunks per core

    xT = nc.dram_tensor("xT", [NB, IN_CH, 128], f32, kind="ExternalInput")
    w1 = nc.dram_tensor("w1", [IN_CH, HID_CH], f32, kind="ExternalInput")
    wmuls = nc.dram_tensor("wmuls", [HID_CH, 2 * OUT_CH], f32, kind="ExternalInput")
    b1b = nc.dram_tensor("b1b", [128, HID_CH], f32, kind="ExternalInput")
    bmlb = nc.dram_tensor("bmlb", [128, 2 * OUT_CH], f32, kind="ExternalInput")
    dinvw = nc.dram_tensor("dinvw", [128, NB], f32, kind="ExternalInput")
    iota_in = nc.dram_tensor("iota_in", [128, 128], i32, kind="ExternalInput")
    ident_in = nc.dram_tensor("ident_in", [128, 128], f32, kind="ExternalInput")
    srcw = nc.dram_tensor("srcw", [128, C], i32, kind="ExternalInput")
    dstw = nc.dram_tensor("dstw", [128, C], i32, kind="ExternalInput")
    out = nc.dram_tensor("out", [NL, 2 * OUT_CH], f32, kind="ExternalOutput")

    u1b = nc.dram_tensor("u1b", [NL, HID_CH], f32)
    u1g = nc.dram_tensor("u1g", [N_NODES, HID_CH], f32)
    u2b = nc.dram_tensor("u2b", [NL, HID_CH], f32)
    u2g = nc.dram_tensor("u2g", [N_NODES, HID_CH], f32)

    with tile.TileContext(nc) as tc:
        with (
            tc.tile_pool(name="const", bufs=1) as cp,
            tc.tile_pool(name="slab", bufs=1) as sp,
            tc.tile_pool(name="xload", bufs=4) as xp,
            tc.tile_pool(name="work", bufs=16) as wp,
            tc.tile_pool(name="mt", bufs=16) as mp,
            tc.tile_pool(name="ep", bufs=4) as ep,
            tc.tile_pool(name="psv", bufs=2, space="PSUM") as ppv,
            tc.tile_pool(name="psg", bufs=2, space="PSUM") as ppg,
            tc.tile_pool(name="pst", bufs=2, space="PSUM") as ppt,
            tc.tile_pool(name="pso", bufs=2, space="PSUM") as ppo,
        ):
            # constants
            w1_sb = [cp.tile([128, HID_CH], f32, name=f"w1sb{k}")
                     for k in range(IN_CH // 128)]
            for k in range(IN_CH // 128):
                nc.sync.dma_start(out=w1_sb[k][:],
                                  in_=w1[k * 128:(k + 1) * 128, :])
            wml_sb = cp.tile([HID_CH, 2 * OUT_CH], f32)
            nc.sync.dma_start(out=wml_sb[:], in_=wmuls[:])
            b1_sb = cp.tile([128, HID_CH], f32)
            nc.sync.dma_start(out=b1_sb[:], in_=b1b[:])
            bml_sb = cp.tile([128, 2 * OUT_CH], f32)
            nc.sync.dma_start(out=bml_sb[:], in_=bmlb[:])
            dinv_sb = cp.tile([128, NB], f32)
            nc.sync.dma_start(out=dinv_sb[:], in_=dinvw[:])
            iota = cp.tile([128, 128], i32)
            nc.sync.dma_start(out=iota[:], in_=iota_in[:])
            ident = cp.tile([128, 128], f32)
            nc.sync.dma_start(out=ident[:], in_=ident_in[:])
            src_sb = sp.tile([128, C], i32)
            nc.sync.dma_start(out=src_sb[:], in_=srcw[:])
            dst_sb = sp.tile([128, C], i32)
            nc.sync.dma_start(out=dst_sb[:], in_=dstw[:])

            u1_slab = sp.tile([128, NB * HID_CH], f32)
            u2_slab = sp.tile([128, NB * HID_CH], f32)

            # phase 1: u1 = (x @ W1) * dinv, node-major tiles
            for m in range(NB):
                rows = 128 if m < NB - 1 else LAST_ROWS
                v_ps = ppv.tile([128, HID_CH], f32, tag="v")
                for k in range(IN_CH // 128):
                    xt_t = xp.tile([128, 128], f32, tag="xt")
                    nc.sync.dma_start(
                        out=xt_t[:], in_=xT[m, k * 128:(k + 1) * 128, :],
                    )
                    nc.tensor.matmul(
                        out=v_ps[:], lhsT=xt_t[:], rhs=w1_sb[k][:],
                        start=(k == 0), stop=(k == IN_CH // 128 - 1),
                    )
                u1_m = u1_slab[:, m * HID_CH:(m + 1) * HID_CH]
                nc.vector.tensor_scalar_mul(u1_m, v_ps[:], dinv_sb[:, m:m + 1])
                nc.sync.dma_start(
                    out=u1b[m * 128:m * 128 + rows, :], in_=u1_m[:rows, :]
                )

            nc.gpsimd.collective_compute(
                "AllGather", mybir.AluOpType.bypass,
                replica_groups=[list(range(N_CORES))],
                ins=[u1b[:].opt()], outs=[u1g[:].opt()],
            )

            def aggregation_pass(table, slab, out_slab_or_none):
                """One S() pass + fused epilogue per bucket. Returns nothing;
                writes h/u2 (pass 1) or final outputs (pass 2)."""
                for b in range(NB):
                    rows = 128 if b < NB - 1 else LAST_ROWS
                    g_ps = ppg.tile([128, HID_CH], f32, tag="g")
                    for j in range(cpb):
                        c = b * cpb + j
                        u_t = wp.tile([128, HID_CH], f32, tag="u")
                        nc.gpsimd.indirect_dma_start(
                            out=u_t[:], out_offset=None, in_=table[:],
                            in_offset=bass.IndirectOffsetOnAxis(
                                ap=src_sb[:, c:c + 1], axis=0),
                        )
                        m_t = mp.tile([128, 128], f32, tag="m")
                        nc.vector.tensor_tensor(
                            out=m_t[:],
                            in0=dst_sb[:, c:c + 1].to_broadcast([128, 128]),
                            in1=iota[:], op=mybir.AluOpType.is_equal,
                        )
                        nc.tensor.matmul(
                            out=g_ps[:], lhsT=m_t[:], rhs=u_t[:],
                            start=(j == 0), stop=(j == cpb - 1),
                        )
                    u_self = slab[:, b * HID_CH:(b + 1) * HID_CH]
                    s_t = ep.tile([128, HID_CH], f32, tag="s")
                    nc.vector.tensor_add(out=s_t[:], in0=g_ps[:], in1=u_self)
                    nc.vector.tensor_scalar_mul(s_t[:], s_t[:], dinv_sb[:, b:b + 1])
                    if out_slab_or_none is not None:
                        # pass 1 epilogue: h = relu(s + b1); u2 = h * dinv
                        nc.vector.tensor_add(out=s_t[:], in0=s_t[:], in1=b1_sb[:])
                        nc.scalar.activation(
                            s_t[:], s_t[:], mybir.ActivationFunctionType.Relu)
                        u2_m = out_slab_or_none[:, b * HID_CH:(b + 1) * HID_CH]
                        nc.vector.tensor_scalar_mul(
                            u2_m, s_t[:], dinv_sb[:, b:b + 1])
                        nc.sync.dma_start(
                            out=u2b[b * 128:b * 128 + rows, :], in_=u2_m[:rows, :])
                    else:
                        # pass 2 epilogue: out = t @ Wmuls + biases
                        tT_ps = ppt.tile([HID_CH, 128], f32, tag="tT")
                        nc.tensor.transpose(
                            out=tT_ps[:], in_=s_t[:], identity=ident[:])
                        tT_sb = ep.tile([HID_CH, 128], f32, tag="tTs")
                        nc.scalar.copy(out=tT_sb[:], in_=tT_ps[:])
                        o_ps = ppo.tile([128, 2 * OUT_CH], f32, tag="o")
                        nc.tensor.matmul(
                            out=o_ps[:], lhsT=tT_sb[:], rhs=wml_sb[:],
                            start=True, stop=True,
                        )
                        o_sb = ep.tile([128, 2 * OUT_CH], f32, tag="os")
                        nc.vector.tensor_add(out=o_sb[:], in0=o_ps[:], in1=bml_sb[:])
                        nc.sync.dma_start(
                            out=out[b * 128:b * 128 + rows, :], in_=o_sb[:rows, :])

            aggregation_pass(u1g, u1_slab, u2_slab)

            nc.gpsimd.collective_compute(
                "AllGather", mybir.AluOpType.bypass,
                replica_groups=[list(range(N_CORES))],
                ins=[u2b[:].opt()], outs=[u2g[:].opt()],
            )

            aggregation_pass(u2g, u2_slab, None)

    _fix_incswdge(nc)
    _split_multiwaits(nc)
    return nc


# ---------------------------------------------------------------------------
# Host-side sharding + launch
# ---------------------------------------------------------------------------
_cache = {}


def _prep(x, edge_index, W1, b1, W_mu, b_mu, W_ls, b_ls):
    x = np.asarray(x, np.float32)
    src = np.asarray(edge_index[0], np.int64).astype(np.int32)
    dst = np.asarray(edge_index[1], np.int64).astype(np.int32)

    deg = np.bincount(dst, minlength=N_NODES).astype(np.float32) + 1.0
    dinv = (1.0 / np.sqrt(deg)).astype(np.float32)

    core = dst // NL
    dst_rel = dst - core * NL
    bucket = dst_rel >> 7
    dst128 = dst_rel & 127

    # per (core,bucket) histogram -> uniform chunks-per-bucket
    gb = core.astype(np.int64) * NB + bucket
    counts = np.bincount(gb, minlength=N_CORES * NB)
    cpb = int((counts.max() + 127) // 128)
    C = NB * cpb

    order = np.argsort(gb, kind="stable")
    gb_s = gb[order]
    starts = np.zeros(N_CORES * NB + 1, np.int64)
    np.cumsum(counts, out=starts[1:])
    rank = np.arange(len(order), dtype=np.int64) - starts[gb_s]

    src_w = np.zeros((N_CORES, 128, C), np.int32)
    dst_w = np.full((N_CORES, 128, C), -1, np.int32)
    cc = gb_s // NB
    bb = gb_s % NB
    col = bb * cpb + rank // 128
    row = rank % 128
    src_w[cc, row, col] = src[order]
    dst_w[cc, row, col] = dst128[order]

    # x^T in tile-major layout: xT[c, m] = x_core[m*128:(m+1)*128].T (contig)
    xT = np.zeros((N_CORES, NB, IN_CH, 128), np.float32)
    xs = x.reshape(N_CORES, NL, IN_CH)
    xpad = np.zeros((N_CORES, XT_COLS, IN_CH), np.float32)
    xpad[:, :NL] = xs
    xT[:] = np.transpose(
        xpad.reshape(N_CORES, NB, 128, IN_CH), (0, 1, 3, 2))

    dinvw = np.ones((N_CORES, 128, NB), np.float32)
    dv = dinv.reshape(N_CORES, NL)
    for b in range(NB):
        rows = 128 if b < NB - 1 else LAST_ROWS
        dinvw[:, :rows, b] = dv[:, b * 128:b * 128 + rows]

    wmuls = np.concatenate([np.asarray(W_mu, np.float32),
                            np.asarray(W_ls, np.float32)], axis=1)
    bml = np.concatenate([np.asarray(b_mu, np.float32),
                          np.asarray(b_ls, np.float32)])[None, :]
    in_map_common = {
        "w1": np.asarray(W1, np.float32),
        "wmuls": wmuls,
        "b1b": np.broadcast_to(np.asarray(b1, np.float32)[None, :],
                               (128, HID_CH)).copy(),
        "bmlb": np.broadcast_to(bml, (128, 2 * OUT_CH)).copy(),
        "iota_in": np.broadcast_to(np.arange(128, dtype=np.int32)[None, :],
                                   (128, 128)).copy(),
        "ident_in": np.eye(128, dtype=np.float32),
    }
    in_maps = []
    for c in range(N_CORES):
        m = dict(in_map_common)
        m["xT"] = xT[c]
        m["srcw"] = src_w[c]
        m["dstw"] = dst_w[c]
        m["dinvw"] = dinvw[c]
        in_maps.append(m)
    return cpb, in_maps


def kernel(x, edge_index, W1, b1, W_mu, b_mu, W_ls, b_ls):
    from concourse.bass_utils import run_bass_kernel_spmd

    cpb, in_maps = _prep(x, edge_index, W1, b1, W_mu, b_mu, W_ls, b_ls)
    if cpb not in _cache:
        _cache[cpb] = _build_program(cpb)
    nc = _cache[cpb]
    res = run_bass_kernel_spmd(nc, in_maps, list(range(N_CORES)))
    full = np.concatenate([res.results[c]["out"] for c in range(N_CORES)], axis=0)
    return full[:, :OUT_CH].copy(), full[:, OUT_CH:].copy()
